# revision 1
# baseline (speedup 1.0000x reference)
# Trainium2 Bass kernel for nn_AttentionPropagation (SuperGlue-style bidirectional
# attentional propagation): 6x (1x1conv+BN+ReLU) filters + QK attention with
# softmax over BOTH axes + two aggregations + output filters.
#
# Sharding: 16 (batch, head) units over 8 cores -> each core owns batch b=core//2
# and a contiguous 128-channel (2-head) slice of the filter outputs.  The final
# filters f4/f5 need all 4 heads of a batch, so core pairs {2b, 2b+1} exchange
# their normalized 128-channel halves of add0/add1 with an AllGather; phase 5 is
# then split by OUTPUT channel across the pair (even core computes och 0-127 of
# both f4 and f5, odd core och 128-255; per-core weight slices keep the SPMD
# instruction stream identical) and the host concatenates.
#
# Per-core dataflow (all matmuls contract over the partition dim):
#   q,k   [128d, 2048]  = ReLU(W'.T @ x_t + b)      (BN folded host-side)
#   v0T   [m, 128d], v1T [n, 128d]: out-transposed filter blocks, 4 blocks
#         batched per PSUM rotation (quad-tiled bias via one K=1 ones-matmul)
#   E-str: QK n-block -> exp (ACT, accum=rowsum) -> E bf16 -> U1 += v1T.T @ E
#   F-str: QKT m-block -> exp (accum=colsum)     -> F bf16 -> U0 += v0T.T @ F
#   add0's normalizer chains (recip+reorder+broadcast of rowsums) run DURING
#   the F-stream on gpsimd/DVE; add1's normalizer is built at F-end with a
#   fast PE chain: rec16 -> transpose (identity matmul) -> SBUF flatten DMA
#   -> ones outer-product into PSUM -> DVE multiply.
#   AllGather pair -> af0/af1 full [256, 2048] -> half-och f4+f5 as 4-block
#   batched out-transposed matmuls -> out DMA in 2KB/partition runs to a
#   partition-major [128, 16, 128] layout (host untransposes).

import numpy as np

B, N, M, C = 4, 2048, 2048, 256
H, Dh = 4, 64
EPS = 1e-5
NCORES = 8
LAG = 6

_CACHE = {}


def _build_program():
    from contextlib import ExitStack

    import concourse.bass as bass
    import concourse.tile as tile
    from concourse import bacc, mybir
    from concourse.bass import ts

    f32 = mybir.dt.float32
    bf16 = mybir.dt.bfloat16
    AF = mybir.ActivationFunctionType
    ALU = mybir.AluOpType

    nc = bacc.Bacc(
        "TRN2",
        target_bir_lowering=False,
        debug=False,
        enable_asserts=False,
        num_devices=NCORES,
    )

    # ---- DRAM I/O ----
    # x tensors host-prereshaped to [128, 2*N] so each partition's DMA span is
    # one contiguous 8KB run.
    x1t_d = nc.dram_tensor("x1t", [128, 2 * N], bf16, kind="ExternalInput").ap()
    x2t_d = nc.dram_tensor("x2t", [128, 2 * M], bf16, kind="ExternalInput").ap()
    wq_d = nc.dram_tensor("wq", [128, 2 * 128], bf16, kind="ExternalInput").ap()
    wk_d = nc.dram_tensor("wk", [128, 2 * 128], bf16, kind="ExternalInput").ap()
    wv0_d = nc.dram_tensor("wv0", [128, 2 * 128], bf16, kind="ExternalInput").ap()
    wv1_d = nc.dram_tensor("wv1", [128, 2 * 128], bf16, kind="ExternalInput").ap()
    bq_d = nc.dram_tensor("bq", [128, 1], f32, kind="ExternalInput").ap()
    bk_d = nc.dram_tensor("bk", [128, 1], f32, kind="ExternalInput").ap()
    bv0_d = nc.dram_tensor("bv0", [1, 128], bf16, kind="ExternalInput").ap()
    bv1_d = nc.dram_tensor("bv1", [1, 128], bf16, kind="ExternalInput").ap()
    # per-core och-half slices of the two output filters (+ 4x-tiled biases)
    w4h_d = nc.dram_tensor("w4h", [128, 2 * 128], bf16, kind="ExternalInput").ap()
    w5h_d = nc.dram_tensor("w5h", [128, 2 * 128], bf16, kind="ExternalInput").ap()
    b4h_d = nc.dram_tensor("b4h", [1, 128], bf16, kind="ExternalInput").ap()
    b5h_d = nc.dram_tensor("b5h", [1, 128], bf16, kind="ExternalInput").ap()
    ones_d = nc.dram_tensor("ones", [1, 128], bf16, kind="ExternalInput").ap()
    ones4_d = nc.dram_tensor("ones4", [1, 512], bf16, kind="ExternalInput").ap()
    ident_d = nc.dram_tensor("ident", [128, 128], f32, kind="ExternalInput").ap()
    identb_d = nc.dram_tensor("identb", [128, 128], bf16, kind="ExternalInput").ap()
    out0_d = nc.dram_tensor("out0h", [128, 16 * 128], f32, kind="ExternalOutput").ap()
    out1_d = nc.dram_tensor("out1h", [128, 16 * 128], f32, kind="ExternalOutput").ap()
    cc_in = nc.dram_tensor("cc_in", [256, N], bf16, kind="Internal").ap()
    cc_out = nc.dram_tensor("cc_out", [512, N], bf16, kind="Internal").ap()
    ccp_in = nc.dram_tensor("ccp_in", [128, 16], bf16, kind="Internal").ap()
    ccp_out = nc.dram_tensor("ccp_out", [256, 16], bf16, kind="Internal").ap()
    sc_d = nc.dram_tensor("sc", [4, 128, 16], f32, kind="Internal").ap()

    NB = N // 128  # 16 n-blocks
    MB = M // 128  # 16 m-blocks

    with tile.TileContext(nc) as tc, ExitStack() as ctx:
        const = ctx.enter_context(tc.tile_pool(name="const", bufs=1))
        # x1t/x2t (phase 1) and af0/af1 (phase 4+) share two slots via one tag
        xpool = ctx.enter_context(tc.tile_pool(name="xp", bufs=2))
        qkp = ctx.enter_context(tc.tile_pool(name="qkp", bufs=1))
        vp = ctx.enter_context(tc.tile_pool(name="vp", bufs=1))
        accp = ctx.enter_context(tc.tile_pool(name="accp", bufs=1))
        addp = ctx.enter_context(tc.tile_pool(name="addp", bufs=1))
        bcp = ctx.enter_context(tc.tile_pool(name="bcp", bufs=1))
        stream = ctx.enter_context(tc.tile_pool(name="stream", bufs=2 * LAG + 6))
        opool = ctx.enter_context(tc.tile_pool(name="opool", bufs=3))
        # PSUM: psS = 2 bufs x [128,1024] (2 banks each) ; psU = 1 x [128,2048]
        psS = ctx.enter_context(tc.tile_pool(name="psS", bufs=2, space="PSUM"))
        psU = ctx.enter_context(tc.tile_pool(name="psU", bufs=1, space="PSUM"))

        # ---- inputs: x2t first (k filter unblocks the E-stream), queues
        # spread: sync = x tensors, gpsimd = qkv weights, scalar = the rest
        x1t_sb = xpool.tile([128, 2, N], bf16, tag="xa")
        x2t_sb = xpool.tile([128, 2, M], bf16, tag="xa")
        x2t_v = x2t_d.rearrange("p (a n) -> p a n", a=2)
        x1t_v = x1t_d.rearrange("p (a n) -> p a n", a=2)
        nc.sync.dma_start(x2t_sb[:, 0], x2t_v[:, 0])
        nc.sync.dma_start(x1t_sb[:, 0], x1t_v[:, 0])
        nc.sync.dma_start(x2t_sb[:, 1], x2t_v[:, 1])
        nc.sync.dma_start(x1t_sb[:, 1], x1t_v[:, 1])

        wq_sb = const.tile([128, 2, 128], bf16, tag="wq")
        wk_sb = const.tile([128, 2, 128], bf16, tag="wk")
        wv0_sb = const.tile([128, 2, 128], bf16, tag="wv0")
        wv1_sb = const.tile([128, 2, 128], bf16, tag="wv1")
        w4h_sb = const.tile([128, 2, 128], bf16, tag="w4h")
        w5h_sb = const.tile([128, 2, 128], bf16, tag="w5h")
        for dst, src in (
            (wk_sb, wk_d), (wq_sb, wq_d), (wv1_sb, wv1_d), (wv0_sb, wv0_d),
        ):
            nc.gpsimd.dma_start(dst[:], src.rearrange("p (a d) -> p a d", a=2))
        bq_sb = const.tile([128, 1], f32, tag="bq")
        bk_sb = const.tile([128, 1], f32, tag="bk")
        bv0_sb = const.tile([1, 128], bf16, tag="bv0")
        bv1_sb = const.tile([1, 128], bf16, tag="bv1")
        b4h_sb = const.tile([1, 128], bf16, tag="b4h")
        b5h_sb = const.tile([1, 128], bf16, tag="b5h")
        ones_t = const.tile([1, 128], bf16, tag="ones")
        ident_sb = const.tile([128, 128], f32, tag="ident")
        identb_sb = const.tile([128, 128], bf16, tag="identb")
        ones4_sb = const.tile([1, 512], bf16, tag="ones4")
        for dst, src in (
            (ones_t, ones_d), (ones4_sb, ones4_d), (bk_sb, bk_d), (bq_sb, bq_d),
            (bv1_sb, bv1_d), (bv0_sb, bv0_d), (b4h_sb, b4h_d), (b5h_sb, b5h_d),
        ):
            nc.scalar.dma_start(dst[:], src)
        nc.scalar.dma_start(identb_sb[:], identb_d)
        nc.scalar.dma_start(ident_sb[:], ident_d)
        for dst, src in ((w4h_sb, w4h_d), (w5h_sb, w5h_d)):
            nc.scalar.dma_start(dst[:], src.rearrange("p (a d) -> p a d", a=2))

        # dummy pair-collective: absorbs the one-time CC mesh setup cost off
        # the critical path (content irrelevant, reads uninitialized DRAM)
        nc.gpsimd.collective_compute(
            "AllGather",
            ALU.bypass,
            replica_groups=[[0, 1], [2, 3], [4, 5], [6, 7]],
            ins=[ccp_in],
            outs=[ccp_out],
        )

        # HAM warm-up: full-contraction accumulating matmuls (the clock-gate
        # monitor watches real PE array activity, so K=1 matmuls don't count)
        # through a full 4096-cycle window, flipping the gate to 8/8
        # (2.4 GHz) before phase 1.
        bps = psS.tile([128, 1024], f32, tag="s")
        for i in range(16):
            nc.tensor.matmul(
                bps[:, 0:512], identb_sb[:], x2t_sb[:, 0, 0:512],
                start=(i == 0), stop=(i == 15),
            )

        # ---- phase 1: q/k filter chunks (ACT relu+bias eviction) interleaved
        # with v1t/v0t 4-block rotations (DVE eviction): both evict engines
        # run concurrently and the PE stays dense enough to hold HAM warm.
        q_sb = qkp.tile([128, N], bf16, tag="q")
        k_sb = qkp.tile([128, M], bf16, tag="k")
        v0t_sb = vp.tile([128, MB * 128], bf16, tag="v0t")  # [m-in-block, mb*128+d]
        v1t_sb = vp.tile([128, NB * 128], bf16, tag="v1t")

        def qk_chunk(dst, xt, w, bias, j, on_act):
            ps = psS.tile([128, 1024], f32, tag="s")
            p5 = ps[:, 0:512]
            nc.tensor.matmul(
                p5, w[:, 0],
                xt[:, 0, ts(j, 512)], start=True, stop=False,
            )
            nc.tensor.matmul(
                p5, w[:, 1],
                xt[:, 1, ts(j, 512)], start=False, stop=True,
            )
            if on_act:
                nc.scalar.activation(dst[:, ts(j, 512)], p5, AF.Relu, bias=bias[:])
            else:
                nc.vector.tensor_scalar(
                    dst[:, ts(j, 512)], p5, bias[:], 0.0, op0=ALU.add, op1=ALU.max
                )

        def v_rot(dst, xt, w, brow, g):
            ps = psS.tile([128, 1024], f32, tag="s")
            p4 = ps[:, 0:512]
            for blk in range(4):
                mb = 4 * g + blk
                sub = p4[:, ts(blk, 128)]
                nc.tensor.matmul(
                    sub, xt[:, 0, ts(mb, 128)],
                    w[:, 0], start=True, stop=False,
                )
                nc.tensor.matmul(
                    sub, xt[:, 1, ts(mb, 128)],
                    w[:, 1], start=False, stop=False,
                )
                nc.tensor.matmul(
                    sub, ones_t[:, 0:128], brow[:], start=False, stop=True,
                )
            nc.vector.tensor_scalar_max(dst[:, ts(g, 512)], p4, 0.0)

        for j in range(4):
            qk_chunk(k_sb, x2t_sb, wk_sb, bk_sb, j, on_act=False)
            v_rot(v1t_sb, x1t_sb, wv1_sb, bv1_sb, j)
        for j in range(4):
            qk_chunk(q_sb, x1t_sb, wq_sb, bq_sb, j, on_act=True)
            v_rot(v0t_sb, x2t_sb, wv0_sb, bv0_sb, j)


        # accumulator scratch for softmax sums: col index = h2*16 + blk
        rows_acc = [
            accp.tile([128, 32], f32, tag=f"ra{u}", name=f"rows_acc{u}")
            for u in range(2)
        ]
        cols_acc = [
            accp.tile([128, 32], f32, tag=f"ca{u}", name=f"cols_acc{u}")
            for u in range(2)
        ]

        # ---- phase 2: E-stream (QK -> exp -> U1 accumulate + E store) ----
        # U1 matmuls consume E tiles LAGGED by LAG blocks so the PE never
        # waits on a just-issued exp: its bubbles during the ACT-bound QK/exp
        # stream are filled from the pending queue, keeping the PE dense.
        U1ps = psU.tile([128, M], f32, tag="u")
        pend = []

        def heater(i, ups):
            # full-array REAL-data matmul into an accumulator quadrant: the
            # clock-gate monitor watches actual array toggling (zeros do not
            # register), and every quadrant is reset later by its first
            # start=True accumulation, so the garbage result is harmless.
            u, h2, j = (i >> 2) & 1, (i >> 1) & 1, i & 1
            nc.tensor.matmul(
                ups[64 * u : 64 * u + 64,
                    h2 * 1024 + 512 * j : h2 * 1024 + 512 * (j + 1)],
                v1t_sb[:, 64 * u : 64 * u + 64],
                x2t_sb[:, 0, ts(i % 4, 512)],
                start=True, stop=True,
                tile_position=(0, 64 * u),
            )

        def emit_u1(u, nb, h2, et):
            for j in range(2):
                nc.tensor.matmul(
                    U1ps[64 * u : 64 * u + 64,
                         h2 * 1024 + 512 * j : h2 * 1024 + 512 * (j + 1)],
                    v1t_sb[:, nb * 128 + 64 * u : nb * 128 + 64 * u + 64],
                    et[:, ts(j, 512)],
                    start=(nb == 0), stop=(nb == NB - 1),
                    tile_position=(0, 64 * u),
                )

        add1_sb = addp.tile([128, M], bf16, tag="a1")

        def pop_u1():
            item = pend.pop(0)
            emit_u1(*item)
            if item[0] == 0 and item[1] == NB - 1 and item[2] == 1:
                # u=0 rows of U1 are complete: evict that half early so the
                # end-of-stream WAR gap shrinks
                nc.vector.tensor_copy(add1_sb[0:64, :], U1ps[0:64, :])

        # stream-head warm burst: gapless full-array real-data matmuls that
        # re-enter the fast clock state right where the exp stream begins.
        for i in range(20):
            heater(i, U1ps)

        hidx = 0
        for u in range(2):
            for nb in range(NB):
                qs = q_sb[64 * u : 64 * u + 64, ts(nb, 128)]
                for h2 in range(2):
                    ps = psS.tile([128, 1024], f32, tag="s")
                    for j in range(2):
                        nc.tensor.matmul(
                            ps[:, ts(j, 512)], qs,
                            k_sb[64 * u : 64 * u + 64,
                                 h2 * 1024 + 512 * j : h2 * 1024 + 512 * (j + 1)
                                 ],
                            start=True, stop=True,
                        )
                    et = stream.tile([128, 1024], bf16, tag="st")
                    nc.scalar.activation(
                        et[:], ps[:], AF.Exp, scale=0.125,
                        accum_out=rows_acc[u][:, h2 * 16 + nb : h2 * 16 + nb + 1],
                    )
                    pend.append((u, nb, h2, et))
                    if len(pend) > LAG:
                        pop_u1()
                    else:
                        heater(hidx, U1ps)
                        hidx += 1
        # the remaining LAG tiles drain INSIDE the F-stream below, so the PE
        # never sees an idle window at the boundary (HAM stays warm)

        # ---- softmax normalizer chain (gpsimd/DMA flavor, for add0: its
        # work overlaps the whole F-stream below) ----
        def norm_chain(dirn, u, acc):
            s16 = accp.tile(
                [128, 16], f32, tag=f"s16_{dirn}{u}", name=f"s16_{dirn}{u}"
            )
            nc.vector.tensor_add(s16[:], acc[u][:, 0:16], acc[u][:, 16:32])
            rec16 = accp.tile([128, 16], f32, tag=f"r16_{dirn}{u}", name=f"r16_{dirn}{u}")
            nc.vector.reciprocal(rec16[:], s16[:])
            # cross-partition reorder via DRAM: flat[0, 128*i + p] = rec16[p, i]
            scs = sc_d[2 * dirn + u]
            nc.gpsimd.dma_start(scs, rec16[:])
            flat = accp.tile([1, 2048], f32, tag=f"fl_{dirn}{u}", name=f"fl_{dirn}{u}")
            nc.gpsimd.dma_start(flat[:], scs.rearrange("p i -> i p"))
            rbc = bcp.tile([128, 2048], f32, tag=f"bc_{dirn}{u}", name=f"bc_{dirn}{u}")
            nc.gpsimd.partition_broadcast(rbc[:], flat[:])
            return rbc

        rbc0 = [norm_chain(0, u, rows_acc) for u in range(2)]

        # ---- phase 3: F-stream (QKT -> exp -> U0, lagged like phase 2).
        # The E-stream's leftover U1 pops are interleaved into the first F
        # tiles; U0ps is allocated only after the last U1 access (the pool
        # rotation reuses U1ps's banks).
        U0ps = None
        pend0 = []

        def emit_u0(u, mb, h2, ft):
            for j in range(2):
                nc.tensor.matmul(
                    U0ps[64 * u : 64 * u + 64,
                         h2 * 1024 + 512 * j : h2 * 1024 + 512 * (j + 1)],
                    v0t_sb[:, mb * 128 + 64 * u : mb * 128 + 64 * u + 64],
                    ft[:, ts(j, 512)],
                    start=(mb == 0), stop=(mb == MB - 1),
                    tile_position=(0, 64 * u),
                )

        for u in range(2):
            for mb in range(MB):
                ks = k_sb[64 * u : 64 * u + 64, ts(mb, 128)]
                for h2 in range(2):
                    ps = psS.tile([128, 1024], f32, tag="s")
                    for j in range(2):
                        nc.tensor.matmul(
                            ps[:, ts(j, 512)], ks,
                            q_sb[64 * u : 64 * u + 64,
                                 h2 * 1024 + 512 * j : h2 * 1024 + 512 * (j + 1)
                                 ],
                            start=True, stop=True,
                        )
                    ft = stream.tile([128, 1024], bf16, tag="st")
                    nc.scalar.activation(
                        ft[:], ps[:], AF.Exp, scale=0.125,
                        accum_out=cols_acc[u][:, h2 * 16 + mb : h2 * 16 + mb + 1],
                    )
                    pend0.append((u, mb, h2, ft))
                    if pend:
                        pop_u1()
                        if not pend:
                            # bridge the u1-half evict gap: garbage matmuls
                            # into the already-evicted u0 rows of U1ps keep
                            # the PE active while the DVE drains rows 64-127
                            for i in range(6):
                                nc.tensor.matmul(
                                    U1ps[0:64, ts(i % 4, 512)],
                                    v1t_sb[0:64, 0:64],
                                    add1_sb[0:64, ts(i % 4, 512)],
                                    start=True, stop=True,
                                )
                            nc.vector.tensor_copy(
                                add1_sb[64:128, :], U1ps[64:128, :]
                            )
                            U0ps = psU.tile([128, N], f32, tag="u")
                            # heater rhs reads the freshly-evicted add1 half:
                            # a real RAW edge that keeps these garbage writes
                            # ordered after the DVE evict of U1ps
                            for i in range(16):
                                u_, h2_, j_ = (i >> 2) & 1, (i >> 1) & 1, i & 1
                                nc.tensor.matmul(
                                    U0ps[64 * u_ : 64 * u_ + 64,
                                         h2_ * 1024 + 512 * j_ : h2_ * 1024 + 512 * (j_ + 1)],
                                    v1t_sb[:, 64 * u_ : 64 * u_ + 64],
                                    add1_sb[:, ts(i % 4, 512)],
                                    start=True, stop=True,
                                    tile_position=(0, 64 * u_),
                                )
                    elif len(pend0) > LAG:
                        emit_u0(*pend0.pop(0))
        for item in pend0:
            emit_u0(*item)

        # ---- normalize: add0 via precomputed rbc0 (DVE); add1 via the fast
        # PE chain (transpose -> flatten -> ones outer-product -> DVE mul)
        add0_sb = addp.tile([128, N], bf16, tag="a0")
        for u in range(2):
            nc.vector.tensor_mul(
                add0_sb[64 * u : 64 * u + 64, :],
                U0ps[64 * u : 64 * u + 64, :],
                rbc0[u][64 * u : 64 * u + 64, :],
            )
        nc.sync.dma_start(cc_in[0:128, :], add0_sb[:])

        for u in range(2):
            s16 = accp.tile([128, 16], f32, tag=f"s16_1{u}", name=f"s16_1{u}")
            nc.vector.tensor_add(s16[:], cols_acc[u][:, 0:16], cols_acc[u][:, 16:32])
            rec16 = accp.tile([128, 16], f32, tag=f"r16_1{u}", name=f"r16_1{u}")
            nc.vector.reciprocal(rec16[:], s16[:])
            psT = psS.tile([128, 1024], f32, tag="s")
            nc.tensor.matmul(psT[0:16, 0:128], rec16[:], ident_sb[:])
            recT = accp.tile([16, 128], bf16, tag=f"rT1{u}", name=f"recT1{u}")
            nc.vector.tensor_copy(recT[:], psT[0:16, 0:128])
            flat = accp.tile([1, 2048], bf16, tag=f"flb1{u}", name=f"flatb1{u}")
            nc.scalar.dma_start(flat[:], recT[:])
            psO = psS.tile([128, 1024], f32, tag="s")
            for j in range(2):
                for jj in range(2):
                    nc.tensor.matmul(
                        psO[:, 512 * jj : 512 * (jj + 1)],
                        ones_t[:, 0:128],
                        flat[0:1, j * 1024 + 512 * jj : j * 1024 + 512 * (jj + 1)],
                    )
                nc.vector.tensor_mul(
                    add1_sb[64 * u : 64 * u + 64, ts(j, 1024)],
                    add1_sb[64 * u : 64 * u + 64, ts(j, 1024)],
                    psO[64 * u : 64 * u + 64, :],
                )
        nc.sync.dma_start(cc_in[128:256, :], add1_sb[:])

        # ---- phase 4: pair AllGather of add0/add1 ----
        nc.gpsimd.collective_compute(
            "AllGather",
            ALU.bypass,
            replica_groups=[[0, 1], [2, 3], [4, 5], [6, 7]],
            ins=[cc_in],
            outs=[cc_out],
        )
        af0 = xpool.tile([128, 2, N], bf16, tag="xa")
        af1 = xpool.tile([128, 2, M], bf16, tag="xa")
        # tiny probe lands first; the burst below re-warms the PE clock gate
        # while the big af halves stream in
        pre_sb = accp.tile([1, 512], bf16, tag="pre", name="pre_sb")
        nc.sync.dma_start(pre_sb[:], cc_out[0:1, 0:512])
        nc.sync.dma_start(af0[:, 0, :], cc_out[0:128, :])
        nc.scalar.dma_start(af0[:, 1, :], cc_out[256:384, :])
        nc.sync.dma_start(af1[:, 0, :], cc_out[128:256, :])
        nc.scalar.dma_start(af1[:, 1, :], cc_out[384:512, :])
        bps2 = psS.tile([128, 1024], f32, tag="s")
        for i in range(16):
            nc.tensor.matmul(
                bps2[:, 0:512], identb_sb[:], af0[:, 0, ts(i % 4, 512)],
                start=(i == 0), stop=(i == 15),
            )

        # ---- phase 5: half-och output filters, 4-block batched rotations,
        # out DMA in 2KB/partition runs to partition-major [128, 16*128]
        for out_d, af, wt, brow, eng in (
            (out0_d, af0, w4h_sb, b4h_sb, nc.sync),
            (out1_d, af1, w5h_sb, b5h_sb, nc.scalar),
        ):
            for g in range(4):
                ps = psS.tile([128, 1024], f32, tag="s")
                p4 = ps[:, 0:512]
                for blk in range(4):
                    nb = 4 * g + blk
                    sub = p4[:, ts(blk, 128)]
                    nc.tensor.matmul(
                        sub, af[:, 0, ts(nb, 128)],
                        wt[:, 0], start=True, stop=False,
                    )
                    nc.tensor.matmul(
                        sub, af[:, 1, ts(nb, 128)],
                        wt[:, 1], start=False, stop=False,
                    )
                    nc.tensor.matmul(
                        sub, ones_t[:, 0:128], brow[:], start=False, stop=True,
                    )
                ot = opool.tile([128, 512], f32, tag="ot")
                nc.vector.tensor_scalar_max(ot[:], p4, 0.0)
                eng.dma_start(out_d[:, ts(g, 512)], ot[:])

    nc.compile()
    return nc


def _prep_core_inputs(inputs):
    """Fold BN into weights, build per-core input maps."""
    x1 = np.ascontiguousarray(inputs["x1"], dtype=np.float32)
    x2 = np.ascontiguousarray(inputs["x2"], dtype=np.float32)
    Ws = np.asarray(inputs["Ws"], dtype=np.float32)
    bs = np.asarray(inputs["bs"], dtype=np.float32)
    g = np.asarray(inputs["gammas"], dtype=np.float32)
    be = np.asarray(inputs["betas"], dtype=np.float32)
    mn = np.asarray(inputs["means"], dtype=np.float32)
    vr = np.asarray(inputs["vars_"], dtype=np.float32)

    s = g / np.sqrt(vr + EPS)  # [6, C]
    Wf = Ws * s[:, :, None]  # rows scaled
    bf = s * (bs - mn) + be

    import ml_dtypes

    bfl = ml_dtypes.bfloat16

    def fold128(a):  # [C, X] -> [128, 2*X] partition-contiguous
        X = a.shape[1]
        return np.ascontiguousarray(
            a.reshape(2, 128, X).transpose(1, 0, 2).reshape(128, 2 * X)
        )

    WfT = np.ascontiguousarray(np.swapaxes(Wf, 1, 2)).astype(bfl)  # [6, C, C]
    x1t = np.ascontiguousarray(np.swapaxes(x1, 1, 2)).astype(bfl)  # [B, C, N]
    x2t = np.ascontiguousarray(np.swapaxes(x2, 1, 2)).astype(bfl)
    bfb = bf.astype(bfl)
    ident = np.eye(128, dtype=np.float32)

    in_maps = []
    for core in range(NCORES):
        b, par = core // 2, core % 2
        sl = slice(par * 128, par * 128 + 128)
        in_maps.append(
            {
                "x1t": fold128(x1t[b]),
                "x2t": fold128(x2t[b]),
                "wq": fold128(WfT[0][:, sl]),
                "wk": fold128(WfT[1][:, sl]),
                "wv0": fold128(WfT[2][:, sl]),
                "wv1": fold128(WfT[3][:, sl]),
                "bq": np.ascontiguousarray(bf[0][sl]).reshape(128, 1),
                "bk": np.ascontiguousarray(bf[1][sl]).reshape(128, 1),
                "bv0": np.ascontiguousarray(bfb[2][sl]).reshape(1, 128),
                "bv1": np.ascontiguousarray(bfb[3][sl]).reshape(1, 128),
                "w4h": fold128(WfT[4][:, sl]),
                "w5h": fold128(WfT[5][:, sl]),
                "b4h": np.ascontiguousarray(bfb[4][sl]).reshape(1, 128),
                "b5h": np.ascontiguousarray(bfb[5][sl]).reshape(1, 128),
                "ones": np.ones((1, 128), bfl),
                "ones4": np.ones((1, 512), bfl),
                "ident": ident,
                "identb": ident.astype(bfl),
            }
        )
    return in_maps


def _gather_outputs(results):
    """results[core][out{0,1}h] is [128, 16*128] partition-major; un-transpose
    to [2048, 128] and concat the pair's och halves."""

    def unfold(a):  # [128, 2048] -> [2048(n), 128(och)]
        return (
            np.ascontiguousarray(
                a.reshape(128, 16, 128).transpose(1, 0, 2)
            ).reshape(2048, 128)
        )

    out0 = np.stack(
        [
            np.concatenate(
                [unfold(results[2 * b]["out0h"]), unfold(results[2 * b + 1]["out0h"])],
                axis=1,
            )
            for b in range(B)
        ]
    )
    out1 = np.stack(
        [
            np.concatenate(
                [unfold(results[2 * b]["out1h"]), unfold(results[2 * b + 1]["out1h"])],
                axis=1,
            )
            for b in range(B)
        ]
    )
    return out0, out1


def kernel(**inputs):
    from concourse import bass_utils

    if "nc" not in _CACHE:
        _CACHE["nc"] = _build_program()
    nc = _CACHE["nc"]

    in_maps = _prep_core_inputs(inputs)
    res = bass_utils.run_bass_kernel_spmd(
        nc, in_maps, core_ids=list(range(NCORES))
    )
    return _gather_outputs(res.results)

